# revision 1
# baseline (speedup 1.0000x reference)
"""GCNConv (PyG-style, alpha-blended residual) on 8 Trainium2 NeuronCores.

Strategy (graph/data parallel, zero collectives):
  out = a*x + (1-a)*(Ahat @ x @ W.T + b)        (aggregate-first form)
The 391 natural 128-destination-node groups are load-balanced across the 8
cores (sorted by chunk count, dealt 8 per slot, so the static per-slot chunk
counts shared by the SPMD program are tight). The full x table is resident in
every core's HBM, so cross-partition "halo" reads are plain local gathers.
Per slot (one 128-dst-node group per core):
  - dma_gather pulls the (dst-sorted, padded) source rows for the group's
    edges into SBUF, 128 edges per chunk; calls are split to <=7 chunks so
    2-3 calls fit in the 128-entry SWDGE descriptor ring and descriptor
    generation overlaps draining. int16 gather indices only reach 32767, so
    the x table is addressed as two 25000-row halves.
  - DVE builds all selection matrices S[e, c, n] = (iota[n]==dstoff[e,c]) *
    norm[e,c] for the slot in two broadcast tensor_tensor ops.
  - PE matmuls S_c^T @ Xg_c accumulate the segment sum in PSUM (float32r:
    full-rate fp32 path), transpose agg, and apply (1-a)*W.T.
  - The preblended residual (a*x + (1-a)*b) is added during the PSUM->SBUF
    copy on DVE (exact fp32).
Degrees / normalization / edge sorting are graph preprocessing done host-side
(pure numpy) — standard practice: the graph structure is static across layers.
"""

import numpy as np

import concourse.bacc as bacc
import concourse.bass as bass
import concourse.mybir as mybir
import concourse.tile as tile
from concourse.bass_utils import run_bass_kernel_spmd

N_NODES = 50000
D = 256
M_CORES = 8
P = 128
HALF = 25000
NG = (N_NODES + P - 1) // P         # 391 natural dst groups
SLOTS = (NG + M_CORES - 1) // M_CORES  # 49 slots per core
MAX_CALL = 10                       # chunks per dma_gather call (ring overlap)

F32 = mybir.dt.float32
F32R = mybir.dt.float32r
I16 = mybir.dt.int16


def _split_call(c):
    out = []
    while c > 0:
        if c <= MAX_CALL:
            out.append(c)
            break
        take = min(MAX_CALL, (c + 1) // 2)
        out.append(take)
        c -= take
    return out


def _preprocess(node_features, edge_index, W, b, alpha):
    x = np.ascontiguousarray(np.asarray(node_features, dtype=np.float32))
    ei = np.asarray(edge_index)
    a = float(np.asarray(alpha).reshape(-1)[0])
    Wf = np.asarray(W, dtype=np.float32)
    bf = np.asarray(b, dtype=np.float32)

    src = ei[0].astype(np.int64)
    dst = ei[1].astype(np.int64)

    deg = (np.bincount(dst, minlength=N_NODES) + 1).astype(np.float32)
    dinv = (1.0 / np.sqrt(deg)).astype(np.float32)  # deg >= 1 (self loops)
    nrm = dinv[src] * dinv[dst]
    dinv2 = dinv * dinv

    gg = dst // P
    doff = (dst - gg * P).astype(np.float32)
    halfb = (src >= HALF).astype(np.int64)
    key = gg * 2 + halfb

    cnt = np.bincount(key, minlength=NG * 2)
    c0 = -(-cnt[0::2] // P)
    c1 = -(-cnt[1::2] // P)

    # deal groups into slots of 8; sort keys on raw per-half edge counts so
    # the per-slot maxes (what the Q7 gather actually pays for) stay tight
    nn0 = cnt[0::2].astype(np.int64)
    nn1 = cnt[1::2].astype(np.int64)
    best = None
    for skey in (
        np.maximum(nn0, nn1) * 4096 + nn0 + nn1,
        nn0 * 4096 + nn1,
        nn1 * 4096 + nn0,
        nn0 + nn1,
    ):
        order = np.argsort(-skey, kind="stable")
        tot = 0
        for r in range(SLOTS):
            blk = order[r * M_CORES:(r + 1) * M_CORES]
            tot += int(nn0[blk].max()) + int(nn1[blk].max())
        if best is None or tot < best[0]:
            best = (tot, order)
    order = best[1]
    assign = np.full((M_CORES, SLOTS), -1, dtype=np.int64)
    core_of = np.zeros(NG, dtype=np.int64)
    slot_of = np.zeros(NG, dtype=np.int64)
    for r in range(SLOTS):
        blk = order[r * M_CORES:(r + 1) * M_CORES]
        for c, g in enumerate(blk):
            assign[c, r] = g
            core_of[g] = c
            slot_of[g] = r

    C0r = np.zeros(SLOTS, dtype=np.int64)
    C1r = np.zeros(SLOTS, dtype=np.int64)
    n0m = np.zeros(SLOTS, dtype=np.int64)
    n1m = np.zeros(SLOTS, dtype=np.int64)
    for r in range(SLOTS):
        blk = assign[:, r]
        blk = blk[blk >= 0]
        C0r[r] = int(c0[blk].max())
        C1r[r] = int(c1[blk].max())
        n0m[r] = int(cnt[0::2][blk].max())
        n1m[r] = int(cnt[1::2][blk].max())
    Cr = C0r + C1r + 1                      # +1: self-loop chunk (no gather)
    cofs = np.concatenate([[0], np.cumsum(Cr)[:-1]])
    TOT = int(Cr.sum())

    # fill per-core edge slot arrays (gathered chunks only)
    eorder = np.argsort(key, kind="stable")
    ks = key[eorder]
    ss = src[eorder]
    nn = nrm[eorder]
    do = doff[eorder]
    starts = np.concatenate([[0], np.cumsum(cnt)[:-1]])
    pos = np.arange(ks.shape[0], dtype=np.int64) - starts[ks]

    g_e = ks // 2
    ch_e = ks % 2
    cr_e = core_of[g_e]
    slot_e = slot_of[g_e]
    base_chunk = cofs[slot_e] + ch_e * C0r[slot_e]
    slot_pos = base_chunk * P + pos

    idx_arr = np.zeros((M_CORES, TOT * P), dtype=np.int16)
    nrm_arr = np.zeros((M_CORES, TOT * P), dtype=np.float32)
    off_arr = np.zeros((M_CORES, TOT * P), dtype=np.float32)
    idx_arr[cr_e, slot_pos] = (ss - ch_e * HALF).astype(np.int16)
    nrm_arr[cr_e, slot_pos] = nn
    off_arr[cr_e, slot_pos] = do

    # self-loop chunk (last chunk of each slot): S = diag(dinv^2), Xg from a
    # sequential per-core slab — saves the per-row Q7 descriptor cost.
    xself_sl = []
    for c in range(M_CORES):
        slab = np.zeros((SLOTS * P, D), dtype=np.float32)
        for r in range(SLOTS):
            g = assign[c, r]
            if g < 0:
                continue
            lo = g * P
            hi = min(lo + P, N_NODES)
            n = hi - lo
            slab[r * P: r * P + n] = x[lo:hi]
            kself = (cofs[r] + Cr[r] - 1) * P
            nrm_arr[c, kself: kself + n] = dinv2[lo:hi]
            off_arr[c, kself: kself + P] = np.arange(P, dtype=np.float32)
        xself_sl.append(slab)

    gidx = [
        np.tile(idx_arr[c].reshape(TOT * 8, 16).T, (8, 1)) for c in range(M_CORES)
    ]
    nrm_in = [np.ascontiguousarray(nrm_arr[c].reshape(TOT, P).T) for c in range(M_CORES)]
    off_in = [np.ascontiguousarray(off_arr[c].reshape(TOT, P).T) for c in range(M_CORES)]

    # preblended residual slabs in slot order; folded weight (1-a)*W.T
    xres_sl = []
    for c in range(M_CORES):
        slab = np.zeros((SLOTS * P, D), dtype=np.float32)
        for r in range(SLOTS):
            g = assign[c, r]
            if g < 0:
                continue
            lo = g * P
            hi = min(lo + P, N_NODES)
            slab[r * P: r * P + hi - lo] = a * x[lo:hi] + (1.0 - a) * bf[None, :]
        xres_sl.append(slab)
    wtp = np.ascontiguousarray(((1.0 - a) * Wf.T).astype(np.float32))

    iota = np.tile(np.arange(P, dtype=np.float32), (P, 1))
    ident = np.eye(P, dtype=np.float32)

    meta = dict(C0r=C0r, C1r=C1r, n0m=n0m, n1m=n1m, cofs=cofs, TOT=TOT, assign=assign)
    return x, gidx, nrm_in, off_in, xres_sl, xself_sl, wtp, iota, ident, meta


def _build(meta):
    C0r, C1r, cofs, TOT = meta["C0r"], meta["C1r"], meta["cofs"], meta["TOT"]
    n0m, n1m = meta["n0m"], meta["n1m"]
    nc = bacc.Bacc("TRN2", debug=False)

    xtab = nc.dram_tensor("xtab", [N_NODES, D], F32R, kind="ExternalInput")
    xres = nc.dram_tensor("xres", [SLOTS * P, D], F32, kind="ExternalInput")
    xself = nc.dram_tensor("xself", [SLOTS * P, D], F32R, kind="ExternalInput")
    gidx = nc.dram_tensor("gidx", [P, TOT * 8], I16, kind="ExternalInput")
    nrmv = nc.dram_tensor("nrmv", [P, TOT], F32R, kind="ExternalInput")
    dofv = nc.dram_tensor("dofv", [P, TOT], F32R, kind="ExternalInput")
    wtp = nc.dram_tensor("wtp", [2 * P, D], F32R, kind="ExternalInput")
    iota = nc.dram_tensor("iota", [P, P], F32R, kind="ExternalInput")
    ident = nc.dram_tensor("ident", [P, P], F32, kind="ExternalInput")
    out = nc.dram_tensor("out", [SLOTS * P, D], F32, kind="ExternalOutput")

    with tile.TileContext(nc) as tc:
        with (
            tc.tile_pool(name="const", bufs=1) as cpool,
            tc.tile_pool(name="xg", bufs=3) as xg_pool,
            tc.tile_pool(name="sel", bufs=3) as s_pool,
            tc.tile_pool(name="sb", bufs=3) as sb_pool,
            tc.tile_pool(name="io", bufs=3) as io_pool,
            tc.tile_pool(name="pagg", bufs=2, space="PSUM") as pagg_pool,
            tc.tile_pool(name="pt", bufs=2, space="PSUM") as pt_pool,
            tc.tile_pool(name="pout", bufs=2, space="PSUM") as pout_pool,
        ):
            iota_sb = cpool.tile([P, P], F32R)
            ident_sb = cpool.tile([P, P], F32)
            wtp0_sb = cpool.tile([P, D], F32R)
            wtp1_sb = cpool.tile([P, D], F32R)
            s0c = int(cofs[1]) * 8      # slot-0 index columns: tiny DMA
            gidx0_sb = cpool.tile([P, s0c], I16)
            gidxR_sb = cpool.tile([P, TOT * 8 - s0c], I16)
            nrm_sb = cpool.tile([P, TOT], F32R)
            dof_sb = cpool.tile([P, TOT], F32R)
            nc.sync.dma_start(out=gidx0_sb[:], in_=gidx[:, 0:s0c])
            nc.sync.dma_start(out=gidxR_sb[:], in_=gidx[:, s0c:TOT * 8])
            nc.sync.dma_start(out=nrm_sb[:], in_=nrmv[:])
            nc.sync.dma_start(out=dof_sb[:], in_=dofv[:])
            nc.sync.dma_start(out=iota_sb[:], in_=iota[:])
            nc.sync.dma_start(out=ident_sb[:], in_=ident[:])
            nc.sync.dma_start(out=wtp0_sb[:], in_=wtp[0:P, :])
            nc.sync.dma_start(out=wtp1_sb[:], in_=wtp[P:2 * P, :])

            CMAX = int((C0r + C1r).max()) + 1

            for r in range(SLOTS):
                C0, C1 = int(C0r[r]), int(C1r[r])
                C = C0 + C1 + 1
                co = int(cofs[r])

                xg = xg_pool.tile([P, CMAX, D], F32R, tag="xg")
                if r < 3:
                    # rotating gather bufs start uninitialized; zero them so
                    # stale tails (masked by zero-norm S lanes) stay finite
                    nc.vector.memset(xg[:].bitcast(F32), 0.0)
                cc0 = 0
                for base, tab_ap, n_chunks, n_exact in (
                    (0, xtab[0:HALF, :], C0, int(n0m[r])),
                    (C0, xtab[HALF:N_NODES, :], C1, int(n1m[r])),
                ):
                    done = 0
                    for n_ch in _split_call(n_chunks):
                        ni = min(n_ch * P, n_exact - done * P)
                        if ni <= 0:
                            break
                        if r == 0:
                            gsl = gidx0_sb[:, cc0 * 8:cc0 * 8 + (ni + 15) // 16]
                        else:
                            gb = (co + cc0) * 8 - s0c
                            gsl = gidxR_sb[:, gb:gb + (ni + 15) // 16]
                        nc.gpsimd.dma_gather(
                            xg[:, cc0:cc0 + n_ch, :],
                            tab_ap,
                            gsl,
                            ni, ni, D, single_packet=False,
                        )
                        cc0 += n_ch
                        done += n_ch
                    cc0 = base + n_chunks if base == 0 else cc0
                cc0 = C0 + C1
                nc.sync.dma_start(
                    out=xg[:, C - 1, :], in_=xself[r * P:(r + 1) * P, :]
                )

                s_all = s_pool.tile([P, CMAX, P], F32R, tag="sel")
                iota_b = iota_sb[:].rearrange("p (c j) -> p c j", c=1).to_broadcast([P, C, P])
                dof_b = dof_sb[:, co:co + C].to_broadcast([P, C, P])
                nrm_b = nrm_sb[:, co:co + C].to_broadcast([P, C, P])
                nc.vector.tensor_tensor(
                    out=s_all[:, 0:C, :], in0=iota_b, in1=dof_b,
                    op=mybir.AluOpType.is_equal,
                )
                nc.vector.tensor_tensor(
                    out=s_all[:, 0:C, :], in0=s_all[:, 0:C, :], in1=nrm_b,
                    op=mybir.AluOpType.mult,
                )

                pagg = pagg_pool.tile([P, D], F32)
                for cc in range(C):
                    nc.tensor.matmul(
                        pagg[:],
                        lhsT=s_all[:, cc, :],
                        rhs=xg[:, cc, :],
                        start=(cc == 0),
                        stop=(cc == C - 1),
                    )

                agg_sb = sb_pool.tile([P, D], F32, tag="agg")
                nc.scalar.copy(agg_sb[:], pagg[:])

                aggT_sb = sb_pool.tile([P, D], F32R, tag="aggT")
                for kb in range(2):
                    pt = pt_pool.tile([P, P], F32)
                    nc.tensor.transpose(
                        pt[:], agg_sb[:, kb * P:(kb + 1) * P], ident_sb[:]
                    )
                    nc.scalar.copy(aggT_sb[:, kb * P:(kb + 1) * P], pt[:])

                xres_sb = io_pool.tile([P, D], F32, tag="xres")
                nc.sync.dma_start(out=xres_sb[:], in_=xres[r * P:(r + 1) * P, :])

                pout = pout_pool.tile([P, D], F32)
                nc.tensor.matmul(
                    pout[:], lhsT=aggT_sb[:, 0:P],
                    rhs=wtp0_sb[:], start=True, stop=False,
                )
                nc.tensor.matmul(
                    pout[:], lhsT=aggT_sb[:, P:2 * P],
                    rhs=wtp1_sb[:], start=False, stop=True,
                )

                out_sb = io_pool.tile([P, D], F32, tag="out")
                nc.vector.tensor_tensor(
                    out=out_sb[:], in0=pout[:], in1=xres_sb[:],
                    op=mybir.AluOpType.add,
                )
                nc.sync.dma_start(out=out[r * P:(r + 1) * P, :], in_=out_sb[:])

    nc.compile()
    return nc


def kernel(node_features, edge_index, W, b, alpha):
    (x, gidx, nrm_in, off_in, xres_sl, xself_sl, wtp, iota, ident, meta) = _preprocess(
        node_features, edge_index, W, b, alpha
    )
    nc = _build(meta)
    in_maps = [
        {
            "xtab": x,
            "xres": xres_sl[c],
            "xself": xself_sl[c],
            "gidx": gidx[c],
            "nrmv": nrm_in[c],
            "dofv": off_in[c],
            "wtp": wtp,
            "iota": iota,
            "ident": ident,
        }
        for c in range(M_CORES)
    ]
    res = run_bass_kernel_spmd(nc, in_maps, list(range(M_CORES)))
    assign = meta["assign"]
    outf = np.empty((N_NODES, D), dtype=np.float32)
    for c in range(M_CORES):
        slab = res.results[c]["out"]
        for r in range(SLOTS):
            g = int(assign[c, r])
            if g < 0:
                continue
            lo = g * P
            hi = min(lo + P, N_NODES)
            outf[lo:hi] = slab[r * P: r * P + hi - lo]
    return outf

